# revision 12
# baseline (speedup 1.0000x reference)
"""AttEdgeConv (dynamic kNN EdgeConv + SE + spatial attention) Trainium2 kernel.

Data-parallel over the 16 point clouds: 8 NeuronCores x 2 clouds each.
Per cloud (N=2048 points, C=64 channels, K=20 neighbors):
  scores s_ij = 2*x_i.x_j - |x_j|^2  (row-monotone with -dist)  via PE matmul (f32r)
  top-20 per row via DVE max8/max_index/match_replace rounds
  edge MLP: pre1 = U_i + V_j; V gathered bf16 via indirect DMA, transposed on PE
  with accumulated U^T (bf16); W2/W3 block-diagonal bf16 on PE; max over K on DVE
  straight out of PSUM; SE + conv attention on Scalar/PE.
"""

import numpy as np

import concourse.bass as bass
import concourse.mybir as mybir
from concourse import bacc
from concourse.masks import make_identity
from concourse.tile import TileContext

B, N, C, K = 16, 2048, 64, 20
NB = 2  # clouds per core
NCORES = 8
NBLK = N // 128  # 16 point-blocks per cloud
KP = K // 2  # 10 k-pair planes
F32 = mybir.dt.float32
F32R = mybir.dt.float32r
BF16 = mybir.dt.bfloat16
U32 = mybir.dt.uint32
AF = mybir.ActivationFunctionType
ALU = mybir.AluOpType
AX = mybir.AxisListType
NEG = -3.0e38


def build(nb=NB, debug=False):
    nc = bacc.Bacc("TRN2", target_bir_lowering=False)
    x_in = nc.dram_tensor("x", [nb * N, C], F32, kind="ExternalInput")
    # w1db cols 0:64 = (W1a - W1b)/2 (+b1 in row 64), cols 64:128 = W1b/2
    w1db_d = nc.dram_tensor("w1db", [C + 1, 2 * C], F32, kind="ExternalInput")
    w2_d = nc.dram_tensor("w2", [C, C], F32, kind="ExternalInput")
    w3_d = nc.dram_tensor("w3", [C, C], F32, kind="ExternalInput")
    b2_d = nc.dram_tensor("b2", [C, 1], F32, kind="ExternalInput")
    b3_d = nc.dram_tensor("b3", [C, 1], F32, kind="ExternalInput")
    b3n_d = nc.dram_tensor("b3n", [C, 1], F32, kind="ExternalInput")  # b3 * N
    se1_d = nc.dram_tensor("se1", [C, C // 16], F32, kind="ExternalInput")  # pre /N
    se2_d = nc.dram_tensor("se2", [C // 16, C], F32, kind="ExternalInput")
    saw_d = nc.dram_tensor("saw", [14, 1], F32, kind="ExternalInput")  # avg/64, max
    out_d = nc.dram_tensor("out", [nb * N, C], F32, kind="ExternalOutput")
    if debug:
        dbg = {
            "s0": nc.dram_tensor("d_s0", [128, N], F32, kind="ExternalOutput"),
            "idx0": nc.dram_tensor("d_idx0", [128, 24], U32, kind="ExternalOutput"),
            "tg0": nc.dram_tensor("d_tg0", [128, K * C], F32, kind="ExternalOutput"),
            "h10": nc.dram_tensor("d_h10", [128, KP * 128], BF16, kind="ExternalOutput"),
            "agg": nc.dram_tensor("d_agg", [C, N], F32, kind="ExternalOutput"),
        }

    with TileContext(nc) as tc:
        with (
            tc.tile_pool(name="const", bufs=1) as constp,
            tc.tile_pool(name="xe", bufs=2) as xep,
            tc.tile_pool(name="xt", bufs=2) as xtp,
            tc.tile_pool(name="xt2", bufs=1) as xt2p,
            tc.tile_pool(name="uv", bufs=2) as uvp,
            tc.tile_pool(name="ssb", bufs=2) as ssbp,
            tc.tile_pool(name="idx", bufs=3) as idxp,
            tc.tile_pool(name="gath", bufs=3) as gathp,
            tc.tile_pool(name="h", bufs=2) as hp,
            tc.tile_pool(name="post", bufs=1) as postp,
            tc.tile_pool(name="fo", bufs=2) as fop,
            tc.tile_pool(name="big_ps", bufs=2, space="PSUM") as bigp,
            tc.tile_pool(name="tp_ps", bufs=2, space="PSUM") as tpp,
            tc.tile_pool(name="mlp_ps", bufs=2, space="PSUM") as mlpp,
            tc.tile_pool(name="vdr", bufs=2, space="DRAM") as vdrp,
        ):
            # ---- constants ----
            ident = constp.tile([128, 128], F32)
            make_identity(nc, ident[:, :])
            identb = constp.tile([128, 128], BF16)
            nc.vector.tensor_copy(identb[:, :], ident[:, :])
            w1db = constp.tile([C + 1, 2 * C], F32)
            nc.sync.dma_start(w1db[:, :], w1db_d[:, :])
            # block-diagonal bf16 W2/W3: rows 0:64/cols 0:64 and rows 64:128/cols 64:128
            w2f = constp.tile([C, C], F32)
            nc.sync.dma_start(w2f[:, :], w2_d[:, :])
            w3f = constp.tile([C, C], F32)
            nc.sync.dma_start(w3f[:, :], w3_d[:, :])
            w2bd = constp.tile([128, 128], BF16)
            nc.vector.memset(w2bd[:, :], 0.0)
            nc.scalar.activation(w2bd[0:C, 0:C], w2f[:, :], AF.Copy)
            nc.scalar.activation(w2bd[C:128, C:128], w2f[:, :], AF.Copy)
            w3bd = constp.tile([128, 128], BF16)
            nc.vector.memset(w3bd[:, :], 0.0)
            nc.scalar.activation(w3bd[0:C, 0:C], w3f[:, :], AF.Copy)
            nc.scalar.activation(w3bd[C:128, C:128], w3f[:, :], AF.Copy)
            se1 = constp.tile([C, C // 16], F32)
            nc.sync.dma_start(se1[:, :], se1_d[:, :])
            se2 = constp.tile([C // 16, C], F32)
            nc.sync.dma_start(se2[:, :], se2_d[:, :])
            saw = constp.tile([14, 1], F32)
            nc.sync.dma_start(saw[:, :], saw_d[:, :])
            b2dup = constp.tile([128, 1], F32)
            nc.sync.dma_start(b2dup[0:C, :], b2_d[:, :])
            nc.sync.dma_start(b2dup[C:128, :], b2_d[:, :])
            b3c = constp.tile([C, 1], F32)
            nc.sync.dma_start(b3c[:, :], b3_d[:, :])
            b3n = constp.tile([C, 1], F32)
            nc.sync.dma_start(b3n[:, :], b3n_d[:, :])
            ones64 = constp.tile([C, 1], F32)
            nc.vector.memset(ones64[:, :], 1.0)
            ones64f = ones64
            onesrow = constp.tile([1, N], F32)
            nc.vector.memset(onesrow[:, :], 1.0)

            for b in range(nb):
                # ================= PREP =================
                xb = x_in[b * N : (b + 1) * N, :]
                xe = xep.tile([128, NBLK, C], F32)  # [pt, blk, ch]
                nc.sync.dma_start(xe[:, :, :], xb.rearrange("(k p) c -> p k c", p=128))

                # transpose pairs of point-blocks -> XT [65, 2048] variants
                xtl = xtp.tile([C + 1, N], F32, tag="xtl")  # 2*x^T ; row64 = 1
                xtr = xtp.tile([C + 1, N], F32, tag="xtr")  # x^T ; row64 = -sq
                xt2 = xt2p.tile([C, N], F32, tag="xt2")  # x^T squared
                for h in range(2):  # halves of 8 blocks
                    pt = bigp.tile([128, 512], F32, tag="big")
                    for j in range(4):
                        pr = 8 * h + 2 * j
                        nc.tensor.transpose(
                            out=pt[:, 128 * j : 128 * (j + 1)],
                            in_=xe[:, pr : pr + 2, :].rearrange("p a c -> p (a c)"),
                            identity=ident[:, :],
                        )
                    # even blocks from partitions 0:64, odd from 64:128
                    for par, off in ((0, 0), (64, 1)):
                        src = pt[par : par + 64, :].rearrange("c (j n) -> c j n", n=128)
                        dstl = xtl[0:C, 1024 * h : 1024 * (h + 1)].rearrange(
                            "c (j t n) -> c j t n", j=4, t=2
                        )[:, :, off, :]
                        dstr = xtr[0:C, 1024 * h : 1024 * (h + 1)].rearrange(
                            "c (j t n) -> c j t n", j=4, t=2
                        )[:, :, off, :]
                        nc.scalar.activation(dstl, src, AF.Copy, scale=2.0)
                        nc.scalar.activation(dstr, src, AF.Copy)
                nc.scalar.activation(xtl[C : C + 1, :], onesrow[:, :], AF.Copy)
                # squares -> -sq into xtr row 64
                nc.scalar.activation(xt2[:, :], xtr[0:C, :], AF.Square)
                for q in range(4):
                    sl = slice(512 * q, 512 * (q + 1))
                    sqp = bigp.tile([1, 512], F32, tag="big")
                    nc.tensor.matmul(
                        sqp[:, :],
                        lhsT=ones64[:, :],
                        rhs=xt2[:, sl],
                    )
                    nc.scalar.activation(xtr[C : C + 1, sl], sqp[:, :], AF.Copy, scale=-1.0)

                # U (dup bf16) per block -> udup ; V (dup bf16) staged to DRAM
                udup = uvp.tile([128, NBLK, 2 * C], F32)
                vst = uvp.tile([128, NBLK, C], F32, tag="vst")
                for i in range(NBLK):
                    up = bigp.tile([128, 2 * C], F32, tag="big")
                    lhs = xtl[:, 128 * i : 128 * (i + 1)]
                    nc.tensor.matmul(
                        up[:, :], lhsT=lhs, rhs=w1db[:, :]
                    )
                    nc.scalar.activation(
                        udup[:, i, :].rearrange("p (a c) -> p a c", a=2),
                        up[:, 0:C].rearrange("p (a c) -> p a c", a=1).to_broadcast(
                            [128, 2, C]
                        ),
                        AF.Copy,
                    )
                    nc.scalar.activation(vst[:, i, :], up[:, C : 2 * C], AF.Copy)
                v_dr = vdrp.tile([N, C], F32)
                nc.sync.dma_start(
                    v_dr[:, :].rearrange("(k p) c -> p k c", p=128), vst[:, :, :]
                )
                agg = postp.tile([C, N], F32, tag="agg")  # max-aggregated h3 (no b3)

                # ================= PER BLOCK =================
                for i in range(NBLK):
                    # ---- scores (f32r matmuls: full-rate PE) ----
                    s_sb = ssbp.tile([128, N], F32)
                    for h in range(2):
                        sp = bigp.tile([128, 1024], F32, tag="big")
                        for q in range(2):
                            sl = slice(1024 * h + 512 * q, 1024 * h + 512 * (q + 1))
                            nc.tensor.matmul(
                                sp[:, 512 * q : 512 * (q + 1)],
                                lhsT=xtl[:, 128 * i : 128 * (i + 1)],
                                rhs=xtr[:, sl],
                            )
                        nc.scalar.activation(
                            s_sb[:, 1024 * h : 1024 * (h + 1)], sp[:, :], AF.Copy
                        )
                    if debug and b == 0 and i == 0:
                        nc.sync.dma_start(dbg["s0"][:, :], s_sb[:, :])
                    # ---- top-20 ----
                    mt = idxp.tile([128, 24], F32, tag="mt")
                    idx = idxp.tile([128, 24], U32, tag="idx")
                    for r in range(3):
                        sl = slice(8 * r, 8 * (r + 1))
                        nc.vector.max(mt[:, sl], s_sb[:, :])
                        nc.vector.max_index(idx[:, sl], mt[:, sl], s_sb[:, :])
                        if r < 2:
                            nc.vector.match_replace(s_sb[:, :], mt[:, sl], s_sb[:, :], NEG)
                    if debug and b == 0 and i == 0:
                        nc.sync.dma_start(dbg["idx0"][:, :], idx[:, :])
                    # ---- wrapped int16 index list for dma_gather ----
                    # list[n] = idx[n % 128, n // 128]; wrap W[q, s] = list[16s+q]
                    # = idxT[s // 8, 16*(s % 8) + q]
                    idxf = idxp.tile([128, K], F32, tag="idxf")
                    nc.vector.tensor_copy(idxf[:, :], idx[:, 0:K])
                    ip_ps = bigp.tile([K, 128], F32, tag="big")
                    nc.tensor.transpose(
                        out=ip_ps[:, :], in_=idxf[:, :], identity=ident[:, :]
                    )
                    idxt = idxp.tile([K, 128], F32, tag="idxt")
                    nc.scalar.activation(idxt[:, :], ip_ps[:, :], AF.Copy)
                    wta = idxp.tile([128, 16], F32, tag="wta")
                    wtb = idxp.tile([32, 16], F32, tag="wtb")
                    idxt_v = idxt[:, :].rearrange("p (a q) -> p a q", q=16)
                    nc.sync.dma_start(wta[:, :], idxt_v[0:16])
                    nc.sync.dma_start(wtb[:, :], idxt_v[16:K])
                    w_ps = bigp.tile([16, 160], F32, tag="big")
                    nc.tensor.transpose(
                        out=w_ps[:, 0:128], in_=wta[:, :], identity=ident[:, :]
                    )
                    nc.tensor.transpose(
                        out=w_ps[:, 128:160], in_=wtb[:, :], identity=ident[0:32, 0:32]
                    )
                    w16 = gathp.tile([128, 160], mybir.dt.int16, tag="w16")
                    nc.vector.tensor_copy(w16[0:16, :], w_ps[:, :])
                    nc.sync.dma_start(w16[16:32, :], w16[0:16, :])
                    nc.sync.dma_start(w16[32:64, :], w16[0:32, :])
                    nc.sync.dma_start(w16[64:128, :], w16[0:64, :])
                    # ---- gather V rows (bf16 dup), add U^T via accumulating transposes ----
                    tg = gathp.tile([128, K, C], F32)
                    nc.gpsimd.dma_gather(
                        out_ap=tg[:, :, :],
                        in_ap=v_dr[:, :],
                        idxs_ap=w16[:, :],
                        num_idxs=128 * K,
                        num_idxs_reg=128 * K,
                        elem_size=C,
                        single_packet=False,
                    )
                    if debug and b == 0 and i == 0:
                        nc.sync.dma_start(
                            dbg["tg0"][:, :].rearrange("p (a c) -> p a c", c=128),
                            tg[:, :, :],
                        )
                    ud = udup[:, i, :]
                    # ---- transpose k-pairs (+U^T) + relu -> h1 [128, KP*128] bf16 ----
                    h1 = hp.tile([128, KP * 128], BF16, tag="h1")
                    for g in range(5):  # groups of 2 k-pairs
                        tp = tpp.tile([128, 256], F32)
                        for t in range(2):
                            j = 2 * g + t
                            csl = slice(128 * t, 128 * (t + 1))
                            nc.tensor.matmul(
                                tp[:, csl],
                                lhsT=tg[:, 2 * j : 2 * (j + 1), :].rearrange(
                                    "p a c -> p (a c)"
                                ),
                                rhs=ident[:, :],
                                is_transpose=True,
                                start=True,
                                stop=False,
                            )
                            nc.tensor.matmul(
                                tp[:, csl],
                                lhsT=ud,
                                rhs=ident[:, :],
                                is_transpose=True,
                                start=False,
                                stop=True,
                            )
                        nc.scalar.activation(
                            h1[:, 256 * g : 256 * (g + 1)], tp[:, :], AF.Relu
                        )
                    if debug and b == 0 and i == 0:
                        nc.sync.dma_start(dbg["h10"][:, :], h1[:, :])
                    # ---- W2 / W3 (block-diag bf16) / maxpool from PSUM ----
                    h2 = hp.tile([128, KP * 128], BF16, tag="h2")
                    for c0, cw in ((0, 512), (512, 512), (1024, 256)):
                        sl = slice(c0, c0 + cw)
                        p2 = mlpp.tile([128, 512], F32, tag="mlp")
                        nc.tensor.matmul(p2[:, 0:cw], lhsT=w2bd[:, :], rhs=h1[:, sl])
                        nc.scalar.activation(
                            h2[:, sl], p2[:, 0:cw], AF.Relu, bias=b2dup[:, :]
                        )
                        p3 = mlpp.tile([128, 512], F32, tag="mlp")
                        nc.tensor.matmul(p3[:, 0:cw], lhsT=w3bd[:, :], rhs=h2[:, sl])
                        # max(even k-plane, odd k-plane); partition-shift via Scalar
                        h3e = hp.tile([C, 512], F32, tag="h3e")
                        h3o = hp.tile([C, 512], F32, tag="h3o")
                        nc.scalar.activation(h3e[:, 0:cw], p3[0:C, 0:cw], AF.Copy)
                        nc.scalar.activation(h3o[:, 0:cw], p3[C:128, 0:cw], AF.Copy)
                        hm = hp.tile([C, 640], F32, tag="hm")
                        nc.vector.tensor_tensor(
                            hm[:, 0:cw], h3e[:, 0:cw], h3o[:, 0:cw], op=ALU.max
                        )
                        # reduce over the k-pairs in this chunk (cw/128 pairs)
                        nc.vector.reduce_max(
                            agg[:, 128 * i : 128 * (i + 1)]
                            if c0 == 0
                            else hm[:, 512 : 512 + 128],
                            hm[:, 0:cw].rearrange("c (j n) -> c n j", j=cw // 128),
                            axis=AX.X,
                        )
                        if c0 != 0:
                            nc.vector.tensor_tensor(
                                agg[:, 128 * i : 128 * (i + 1)],
                                agg[:, 128 * i : 128 * (i + 1)],
                                hm[:, 512 : 512 + 128],
                                op=ALU.max,
                            )

                # ================= POST =================
                if debug and b == 0:
                    nc.sync.dma_start(dbg["agg"][:, :], agg[:, :])
                # SE channel attention: s = sum(agg)/N + b3 ; se1 pre-divided by N
                s_sum = postp.tile([C, 1], F32, tag="ssum")
                nc.vector.reduce_sum(s_sum[:, :], agg[:, :], axis=AX.X)
                s_in = postp.tile([C, 1], F32, tag="sin")
                nc.vector.tensor_add(s_in[:, :], s_sum[:, :], b3n[:, :])
                se_p = bigp.tile([C // 16, 1], F32, tag="big")
                nc.tensor.matmul(se_p[:, :], lhsT=se1[:, :], rhs=s_in[:, :])
                se_t = postp.tile([C // 16, 1], F32, tag="set")
                nc.scalar.activation(se_t[:, :], se_p[:, :], AF.Relu)
                se_p2 = bigp.tile([C, 1], F32, tag="big")
                nc.tensor.matmul(se_p2[:, :], lhsT=se2[:, :], rhs=se_t[:, :])
                yse = postp.tile([C, 1], F32, tag="yse")
                nc.scalar.activation(yse[:, :], se_p2[:, :], AF.Sigmoid)
                # x_se = (agg + b3) * yse  == agg*yse + (b3*yse)  on Scalar
                b3y = postp.tile([C, 1], F32, tag="b3y")
                nc.vector.tensor_mul(b3y[:, :], b3c[:, :], yse[:, :])
                x_se = postp.tile([C, N], F32, tag="xse")
                nc.scalar.activation(
                    x_se[:, :], agg[:, :], AF.Identity, scale=yse[:, :], bias=b3y[:, :]
                )

                # spatial attention: avg + max over channels, conv7, sigmoid
                rowa = postp.tile([1, N + 6], F32, tag="rowa")
                rowm = postp.tile([1, N + 6], F32, tag="rowm")
                for t in (rowa, rowm):
                    nc.vector.memset(t[:, 0:3], 0.0)
                    nc.vector.memset(t[:, N + 3 : N + 6], 0.0)
                for q in range(4):
                    sl = slice(512 * q, 512 * (q + 1))
                    ap_ = bigp.tile([1, 512], F32, tag="big")
                    nc.tensor.matmul(
                        ap_[:, :],
                        lhsT=ones64f[:, :],
                        rhs=x_se[:, sl],
                    )
                    nc.scalar.activation(
                        rowa[:, 512 * q + 3 : 512 * (q + 1) + 3], ap_[:, :], AF.Copy
                    )
                # transpose x_se blocks once; reused for channel-max and final out
                xsb = postp.tile([128, NBLK, C], F32, tag="xsb")
                for g in range(4):
                    xp = bigp.tile([128, 256], F32, tag="big")
                    for t in range(4):
                        i = 4 * g + t
                        nc.tensor.transpose(
                            out=xp[:, 64 * t : 64 * (t + 1)],
                            in_=x_se[:, 128 * i : 128 * (i + 1)],
                            identity=ident[0:C, 0:C],
                        )
                    nc.scalar.activation(
                        xsb[:, 4 * g : 4 * (g + 1), :].rearrange("p a c -> p (a c)"),
                        xp[:, :],
                        AF.Copy,
                    )
                mcol = postp.tile([128, NBLK], F32, tag="mcol")
                for i in range(NBLK):
                    nc.vector.reduce_max(mcol[:, i : i + 1], xsb[:, i, :], axis=AX.X)
                mrow_dr = vdrp.tile([N], F32)
                nc.sync.dma_start(
                    mrow_dr[:].rearrange("(k p) -> p k", p=128), mcol[:, :]
                )
                nc.sync.dma_start(rowm[:, 3 : N + 3], mrow_dr[:])
                im2c = postp.tile([14, N], F32, tag="im2c")
                for row, t in ((0, rowa), (7, rowm)):
                    for tt in range(7):
                        nc.sync.dma_start(
                            im2c[row + tt : row + tt + 1, :], t[0:1, tt : tt + N]
                        )
                attp_t = postp.tile([1, N + 6], F32, tag="rowa")
                attp = attp_t[:, 0:N]
                for q in range(4):
                    sl = slice(512 * q, 512 * (q + 1))
                    cp = bigp.tile([1, 512], F32, tag="big")
                    nc.tensor.matmul(cp[:, :], lhsT=saw[:, :], rhs=im2c[:, sl])
                    nc.scalar.activation(attp[:, sl], cp[:, :], AF.Copy)
                atc_p = bigp.tile([128, NBLK], F32, tag="big")
                for i in range(NBLK):
                    nc.tensor.transpose(
                        out=atc_p[:, i : i + 1],
                        in_=attp[:, 128 * i : 128 * (i + 1)],
                        identity=ident[0:1, 0:1],
                    )
                atc = postp.tile([128, NBLK], F32, tag="atc")
                nc.scalar.activation(atc[:, :], atc_p[:, :], AF.Sigmoid)

                # final: out = transpose(x_se)*att + x
                fo = fop.tile([128, NBLK, C], F32)
                for i in range(NBLK):
                    nc.vector.scalar_tensor_tensor(
                        fo[:, i, :],
                        in0=xsb[:, i, :],
                        scalar=atc[:, i : i + 1],
                        in1=xe[:, i, :],
                        op0=ALU.mult,
                        op1=ALU.add,
                    )
                nc.sync.dma_start(
                    out_d[b * N : (b + 1) * N, :].rearrange("(k p) c -> p k c", p=128),
                    fo[:, :, :],
                )
    nc.compile()
    return nc


_NC_CACHE = {}


def get_nc():
    if "nc" not in _NC_CACHE:
        _NC_CACHE["nc"] = build()
    return _NC_CACHE["nc"]


def prep_weights(W1, b1, W2, b2, W3, b3, se_w1, se_w2, sa_w):
    W1a, W1b = W1[:C], W1[C:]
    w1d = np.concatenate([(W1a - W1b) / 2.0, b1[None, :]], axis=0).astype(np.float32)
    w1b_ = np.concatenate([W1b / 2.0, np.zeros((1, C), np.float32)], axis=0).astype(
        np.float32
    )
    w1db = np.concatenate([w1d, w1b_], axis=1).astype(np.float32)
    saw = np.concatenate([sa_w[0, 0] / C, sa_w[0, 1]]).astype(np.float32)[:, None]
    return {
        "w1db": w1db,
        "w2": W2.astype(np.float32),
        "w3": W3.astype(np.float32),
        "b2": b2.astype(np.float32)[:, None],
        "b3": b3.astype(np.float32)[:, None],
        "b3n": (b3 * N).astype(np.float32)[:, None],
        "se1": (se_w1 / N).astype(np.float32),
        "se2": se_w2.astype(np.float32),
        "saw": saw,
    }


def kernel(x, batch, batch_size, W1, b1, W2, b2, W3, b3, se_w1, se_w2, sa_w, **kw):
    from concourse.bass_utils import run_bass_kernel_spmd

    x = np.asarray(x, np.float32)
    wts = prep_weights(
        np.asarray(W1, np.float32),
        np.asarray(b1, np.float32),
        np.asarray(W2, np.float32),
        np.asarray(b2, np.float32),
        np.asarray(W3, np.float32),
        np.asarray(b3, np.float32),
        np.asarray(se_w1, np.float32),
        np.asarray(se_w2, np.float32),
        np.asarray(sa_w, np.float32),
    )
    nc = get_nc()
    xr = x.reshape(B, N, C)
    in_maps = []
    for c in range(NCORES):
        m = {"x": np.ascontiguousarray(xr[c * NB : (c + 1) * NB].reshape(NB * N, C))}
        m.update(wts)
        in_maps.append(m)
    res = run_bass_kernel_spmd(nc, in_maps, core_ids=list(range(NCORES)))
    out = np.concatenate([r["out"] for r in res.results], axis=0)
    return out.astype(np.float32)


if __name__ == "__main__":
    nc = build()
    print("built ok")


# revision 17
# speedup vs baseline: 1.1596x; 1.1596x over previous
"""AttEdgeConv (dynamic kNN EdgeConv + SE + spatial attention) Trainium2 kernel.

Data-parallel over the 16 point clouds: 8 NeuronCores x 2 clouds each.
Per cloud (N=2048 points, C=64 channels, K=20 neighbors):
  scores s_ij = 2*x_i.x_j - |x_j|^2  (row-monotone with -dist)  via PE matmul (f32r)
  top-20 per row via DVE max8/max_index/match_replace rounds
  edge MLP: pre1 = U_i + V_j; V gathered bf16 via indirect DMA, transposed on PE
  with accumulated U^T (bf16); W2/W3 block-diagonal bf16 on PE; max over K on DVE
  straight out of PSUM; SE + conv attention on Scalar/PE.
"""

import numpy as np

import concourse.bass as bass
import concourse.mybir as mybir
from concourse import bacc
from concourse.masks import make_identity
from concourse.tile import TileContext

B, N, C, K = 16, 2048, 64, 20
NB = 2  # clouds per core
NCORES = 8
NBLK = N // 128  # 16 point-blocks per cloud
KP = K // 2  # 10 k-pair planes
F32 = mybir.dt.float32
F32R = mybir.dt.float32r
BF16 = mybir.dt.bfloat16
U32 = mybir.dt.uint32
AF = mybir.ActivationFunctionType
ALU = mybir.AluOpType
AX = mybir.AxisListType
NEG = -3.0e38


def build(nb=NB, debug=False):
    nc = bacc.Bacc("TRN2", target_bir_lowering=False)
    x_in = nc.dram_tensor("x", [nb * N, C], F32, kind="ExternalInput")
    # w1db cols 0:64 = (W1a - W1b)/2 (+b1 in row 64), cols 64:128 = W1b/2
    w1db_d = nc.dram_tensor("w1db", [C + 1, 2 * C], F32, kind="ExternalInput")
    w2_d = nc.dram_tensor("w2", [C, C], F32, kind="ExternalInput")
    w3_d = nc.dram_tensor("w3", [C, C], F32, kind="ExternalInput")
    b2_d = nc.dram_tensor("b2", [C, 1], F32, kind="ExternalInput")
    b3_d = nc.dram_tensor("b3", [C, 1], F32, kind="ExternalInput")
    b3n_d = nc.dram_tensor("b3n", [C, 1], F32, kind="ExternalInput")  # b3 * N
    se1_d = nc.dram_tensor("se1", [C, C // 16], F32, kind="ExternalInput")  # pre /N
    se2_d = nc.dram_tensor("se2", [C // 16, C], F32, kind="ExternalInput")
    saw_d = nc.dram_tensor("saw", [14, 1], F32, kind="ExternalInput")  # avg/64, max
    out_d = nc.dram_tensor("out", [nb * N, C], F32, kind="ExternalOutput")
    if debug:
        dbg = {
            "s0": nc.dram_tensor("d_s0", [128, N], F32, kind="ExternalOutput"),
            "idx0": nc.dram_tensor("d_idx0", [128, 24], U32, kind="ExternalOutput"),
            "tg0": nc.dram_tensor("d_tg0", [128, K * C], F32, kind="ExternalOutput"),
            "h10": nc.dram_tensor("d_h10", [128, KP * 128], BF16, kind="ExternalOutput"),
            "agg": nc.dram_tensor("d_agg", [C, N], F32, kind="ExternalOutput"),
        }

    with TileContext(nc) as tc:
        with (
            tc.tile_pool(name="const", bufs=1) as constp,
            tc.tile_pool(name="xe", bufs=2) as xep,
            tc.tile_pool(name="xt", bufs=2) as xtp,
            tc.tile_pool(name="xt2", bufs=1) as xt2p,
            tc.tile_pool(name="uv", bufs=2) as uvp,
            tc.tile_pool(name="ssb", bufs=2) as ssbp,
            tc.tile_pool(name="idx", bufs=3) as idxp,
            tc.tile_pool(name="gath", bufs=3) as gathp,
            tc.tile_pool(name="h", bufs=2) as hp,
            tc.tile_pool(name="post", bufs=1) as postp,
            tc.tile_pool(name="fo", bufs=2) as fop,
            tc.tile_pool(name="big_ps", bufs=2, space="PSUM") as bigp,
            tc.tile_pool(name="tp_ps", bufs=2, space="PSUM") as tpp,
            tc.tile_pool(name="mlp_ps", bufs=2, space="PSUM") as mlpp,
            tc.tile_pool(name="vdr", bufs=2, space="DRAM") as vdrp,
        ):
            # ---- constants ----
            ident = constp.tile([128, 128], F32)
            make_identity(nc, ident[:, :])
            identb = constp.tile([128, 128], BF16)
            nc.vector.tensor_copy(identb[:, :], ident[:, :])
            w1db = constp.tile([C + 1, 2 * C], F32)
            nc.sync.dma_start(w1db[:, :], w1db_d[:, :])
            # block-diagonal bf16 W2/W3: rows 0:64/cols 0:64 and rows 64:128/cols 64:128
            w2f = constp.tile([C, C], F32)
            nc.sync.dma_start(w2f[:, :], w2_d[:, :])
            w3f = constp.tile([C, C], F32)
            nc.sync.dma_start(w3f[:, :], w3_d[:, :])
            w2bd = constp.tile([128, 128], BF16)
            nc.vector.memset(w2bd[:, :], 0.0)
            nc.scalar.activation(w2bd[0:C, 0:C], w2f[:, :], AF.Copy)
            nc.scalar.activation(w2bd[C:128, C:128], w2f[:, :], AF.Copy)
            w3bd = constp.tile([128, 128], BF16)
            nc.vector.memset(w3bd[:, :], 0.0)
            nc.scalar.activation(w3bd[0:C, 0:C], w3f[:, :], AF.Copy)
            nc.scalar.activation(w3bd[C:128, C:128], w3f[:, :], AF.Copy)
            se1 = constp.tile([C, C // 16], F32)
            nc.sync.dma_start(se1[:, :], se1_d[:, :])
            se2 = constp.tile([C // 16, C], F32)
            nc.sync.dma_start(se2[:, :], se2_d[:, :])
            saw = constp.tile([14, 1], F32)
            nc.sync.dma_start(saw[:, :], saw_d[:, :])
            b2dup = constp.tile([128, 1], F32)
            nc.sync.dma_start(b2dup[0:C, :], b2_d[:, :])
            nc.sync.dma_start(b2dup[C:128, :], b2_d[:, :])
            b3c = constp.tile([C, 1], F32)
            nc.sync.dma_start(b3c[:, :], b3_d[:, :])
            b3n = constp.tile([C, 1], F32)
            nc.sync.dma_start(b3n[:, :], b3n_d[:, :])
            ones64 = constp.tile([C, 1], F32)
            nc.vector.memset(ones64[:, :], 1.0)
            ones64f = ones64
            onesrow = constp.tile([1, N], F32)
            nc.vector.memset(onesrow[:, :], 1.0)

            for b in range(nb):
                # ================= PREP =================
                xb = x_in[b * N : (b + 1) * N, :]
                xe = xep.tile([128, NBLK, C], F32)  # [pt, blk, ch]
                nc.sync.dma_start(xe[:, :, :], xb.rearrange("(k p) c -> p k c", p=128))

                # transpose pairs of point-blocks -> XT [65, 2048] variants
                xtl = xtp.tile([C + 1, N], F32, tag="xtl")  # 2*x^T ; row64 = 1
                xtr = xtp.tile([C + 1, N], F32, tag="xtr")  # x^T ; row64 = -sq
                xt2 = xt2p.tile([C, N], F32, tag="xt2")  # x^T squared
                for h in range(2):  # halves of 8 blocks
                    pt = bigp.tile([128, 512], F32, tag="big")
                    for j in range(4):
                        pr = 8 * h + 2 * j
                        nc.tensor.transpose(
                            out=pt[:, 128 * j : 128 * (j + 1)],
                            in_=xe[:, pr : pr + 2, :].rearrange("p a c -> p (a c)"),
                            identity=ident[:, :],
                        )
                    # even blocks from partitions 0:64, odd from 64:128
                    for par, off in ((0, 0), (64, 1)):
                        src = pt[par : par + 64, :].rearrange("c (j n) -> c j n", n=128)
                        dstl = xtl[0:C, 1024 * h : 1024 * (h + 1)].rearrange(
                            "c (j t n) -> c j t n", j=4, t=2
                        )[:, :, off, :]
                        dstr = xtr[0:C, 1024 * h : 1024 * (h + 1)].rearrange(
                            "c (j t n) -> c j t n", j=4, t=2
                        )[:, :, off, :]
                        nc.scalar.activation(dstl, src, AF.Copy, scale=2.0)
                        nc.scalar.activation(dstr, src, AF.Copy)
                nc.scalar.activation(xtl[C : C + 1, :], onesrow[:, :], AF.Copy)
                # squares -> -sq into xtr row 64
                nc.scalar.activation(xt2[:, :], xtr[0:C, :], AF.Square)
                for q in range(4):
                    sl = slice(512 * q, 512 * (q + 1))
                    sqp = bigp.tile([1, 512], F32, tag="big")
                    nc.tensor.matmul(
                        sqp[:, :],
                        lhsT=ones64[:, :],
                        rhs=xt2[:, sl],
                    )
                    nc.scalar.activation(xtr[C : C + 1, sl], sqp[:, :], AF.Copy, scale=-1.0)

                # U (dup bf16) per block -> udup ; V (dup bf16) staged to DRAM
                udup = uvp.tile([128, NBLK, 2 * C], F32)
                vst = uvp.tile([128, NBLK, C], F32, tag="vst")
                for i in range(NBLK):
                    up = bigp.tile([128, 2 * C], F32, tag="big")
                    lhs = xtl[:, 128 * i : 128 * (i + 1)]
                    nc.tensor.matmul(
                        up[:, :], lhsT=lhs, rhs=w1db[:, :]
                    )
                    nc.scalar.activation(
                        udup[:, i, :].rearrange("p (a c) -> p a c", a=2),
                        up[:, 0:C].rearrange("p (a c) -> p a c", a=1).to_broadcast(
                            [128, 2, C]
                        ),
                        AF.Copy,
                    )
                    nc.scalar.activation(vst[:, i, :], up[:, C : 2 * C], AF.Copy)
                v_dr = vdrp.tile([N, C], F32)
                nc.sync.dma_start(
                    v_dr[:, :].rearrange("(k p) c -> p k c", p=128), vst[:, :, :]
                )
                agg = postp.tile([C, N], F32, tag="agg")  # max-aggregated h3 (no b3)

                # ====== PER BLOCK: 3-stage skewed pipeline ======
                # iteration k issues: scores(k+2) | topk+wrap+gather(k+1) | MLP(k)

                def front_scores(i):
                    s_sb = ssbp.tile([128, N], F32)
                    for h in range(2):
                        sp = bigp.tile([128, 1024], F32, tag="big")
                        for q in range(2):
                            sl = slice(1024 * h + 512 * q, 1024 * h + 512 * (q + 1))
                            nc.tensor.matmul(
                                sp[:, 512 * q : 512 * (q + 1)],
                                lhsT=xtl[:, 128 * i : 128 * (i + 1)],
                                rhs=xtr[:, sl],
                            )
                        nc.scalar.activation(
                            s_sb[:, 1024 * h : 1024 * (h + 1)], sp[:, :], AF.Copy
                        )
                    return s_sb

                def front_topk_gather(i, s_sb):
                    mt = idxp.tile([128, 24], F32, tag="mt")
                    idx = idxp.tile([128, 24], U32, tag="idx")
                    for r in range(3):
                        sl = slice(8 * r, 8 * (r + 1))
                        nc.vector.max(mt[:, sl], s_sb[:, :])
                        nc.vector.max_index(idx[:, sl], mt[:, sl], s_sb[:, :])
                        if r < 2:
                            nc.vector.match_replace(s_sb[:, :], mt[:, sl], s_sb[:, :], NEG)
                    idxf = idxp.tile([128, K], F32, tag="idxf")
                    nc.vector.tensor_copy(idxf[:, :], idx[:, 0:K])
                    ip_ps = bigp.tile([K, 128], F32, tag="big")
                    nc.tensor.transpose(
                        out=ip_ps[:, :], in_=idxf[:, :], identity=ident[:, :]
                    )
                    idxt = idxp.tile([K, 128], F32, tag="idxt")
                    nc.scalar.activation(idxt[:, :], ip_ps[:, :], AF.Copy)
                    wta = idxp.tile([128, 16], F32, tag="wta")
                    wtb = idxp.tile([32, 16], F32, tag="wtb")
                    idxt_v = idxt[:, :].rearrange("p (a q) -> p a q", q=16)
                    nc.sync.dma_start(wta[:, :], idxt_v[0:16])
                    nc.sync.dma_start(wtb[:, :], idxt_v[16:K])
                    w_ps = bigp.tile([16, 160], F32, tag="big")
                    nc.tensor.transpose(
                        out=w_ps[:, 0:128], in_=wta[:, :], identity=ident[:, :]
                    )
                    nc.tensor.transpose(
                        out=w_ps[:, 128:160], in_=wtb[:, :], identity=ident[0:32, 0:32]
                    )
                    w16 = gathp.tile([128, 160], mybir.dt.int16, tag="w16")
                    nc.vector.tensor_copy(w16[0:16, :], w_ps[:, :])
                    nc.sync.dma_start(w16[16:32, :], w16[0:16, :])
                    nc.sync.dma_start(w16[32:64, :], w16[0:32, :])
                    nc.sync.dma_start(w16[64:128, :], w16[0:64, :])
                    tg = gathp.tile([128, K, C], F32)
                    nc.gpsimd.dma_gather(
                        out_ap=tg[:, :, :],
                        in_ap=v_dr[:, :],
                        idxs_ap=w16[:, :],
                        num_idxs=128 * K,
                        num_idxs_reg=128 * K,
                        elem_size=C,
                        single_packet=False,
                    )
                    return tg

                def mlp_block(i, tg):
                    ud = udup[:, i, :]
                    h1 = hp.tile([128, KP * 128], BF16, tag="h1")
                    for g in range(5):
                        tp = tpp.tile([128, 256], F32)
                        for t in range(2):
                            j = 2 * g + t
                            csl = slice(128 * t, 128 * (t + 1))
                            nc.tensor.matmul(
                                tp[:, csl],
                                lhsT=tg[:, 2 * j : 2 * (j + 1), :].rearrange(
                                    "p a c -> p (a c)"
                                ),
                                rhs=ident[:, :],
                                is_transpose=True,
                                start=True,
                                stop=False,
                            )
                            nc.tensor.matmul(
                                tp[:, csl],
                                lhsT=ud,
                                rhs=ident[:, :],
                                is_transpose=True,
                                start=False,
                                stop=True,
                            )
                        nc.scalar.activation(
                            h1[:, 256 * g : 256 * (g + 1)], tp[:, :], AF.Relu
                        )
                    h2 = hp.tile([128, KP * 128], BF16, tag="h2")
                    for c0, cw in ((0, 512), (512, 512), (1024, 256)):
                        sl = slice(c0, c0 + cw)
                        p2 = mlpp.tile([128, 512], F32, tag="mlp")
                        nc.tensor.matmul(p2[:, 0:cw], lhsT=w2bd[:, :], rhs=h1[:, sl])
                        nc.scalar.activation(
                            h2[:, sl], p2[:, 0:cw], AF.Relu, bias=b2dup[:, :]
                        )
                        p3 = mlpp.tile([128, 512], F32, tag="mlp")
                        nc.tensor.matmul(p3[:, 0:cw], lhsT=w3bd[:, :], rhs=h2[:, sl])
                        h3e = hp.tile([C, 512], F32, tag="h3e")
                        h3o = hp.tile([C, 512], F32, tag="h3o")
                        nc.scalar.activation(h3e[:, 0:cw], p3[0:C, 0:cw], AF.Copy)
                        nc.scalar.activation(h3o[:, 0:cw], p3[C:128, 0:cw], AF.Copy)
                        hm = hp.tile([C, 640], F32, tag="hm")
                        nc.vector.tensor_tensor(
                            hm[:, 0:cw], h3e[:, 0:cw], h3o[:, 0:cw], op=ALU.max
                        )
                        nc.vector.reduce_max(
                            agg[:, 128 * i : 128 * (i + 1)]
                            if c0 == 0
                            else hm[:, 512 : 512 + 128],
                            hm[:, 0:cw].rearrange("c (j n) -> c n j", j=cw // 128),
                            axis=AX.X,
                        )
                        if c0 != 0:
                            nc.vector.tensor_tensor(
                                agg[:, 128 * i : 128 * (i + 1)],
                                agg[:, 128 * i : 128 * (i + 1)],
                                hm[:, 512 : 512 + 128],
                                op=ALU.max,
                            )

                ssb_d = {0: front_scores(0), 1: front_scores(1)}
                tg_d = {0: front_topk_gather(0, ssb_d.pop(0))}
                for i in range(NBLK):
                    if i + 2 < NBLK:
                        ssb_d[i + 2] = front_scores(i + 2)
                    if i + 1 < NBLK:
                        tg_d[i + 1] = front_topk_gather(i + 1, ssb_d.pop(i + 1))
                    mlp_block(i, tg_d.pop(i))

                # ================= POST =================
                if debug and b == 0:
                    nc.sync.dma_start(dbg["agg"][:, :], agg[:, :])
                # SE channel attention: s = sum(agg)/N + b3 ; se1 pre-divided by N
                s_sum = postp.tile([C, 1], F32, tag="ssum")
                nc.vector.reduce_sum(s_sum[:, :], agg[:, :], axis=AX.X)
                s_in = postp.tile([C, 1], F32, tag="sin")
                nc.vector.tensor_add(s_in[:, :], s_sum[:, :], b3n[:, :])
                se_p = bigp.tile([C // 16, 1], F32, tag="big")
                nc.tensor.matmul(se_p[:, :], lhsT=se1[:, :], rhs=s_in[:, :])
                se_t = postp.tile([C // 16, 1], F32, tag="set")
                nc.scalar.activation(se_t[:, :], se_p[:, :], AF.Relu)
                se_p2 = bigp.tile([C, 1], F32, tag="big")
                nc.tensor.matmul(se_p2[:, :], lhsT=se2[:, :], rhs=se_t[:, :])
                yse = postp.tile([C, 1], F32, tag="yse")
                nc.scalar.activation(yse[:, :], se_p2[:, :], AF.Sigmoid)
                # x_se = (agg + b3) * yse  == agg*yse + (b3*yse)  on Scalar
                b3y = postp.tile([C, 1], F32, tag="b3y")
                nc.vector.tensor_mul(b3y[:, :], b3c[:, :], yse[:, :])
                x_se = postp.tile([C, N], F32, tag="xse")
                nc.scalar.activation(
                    x_se[:, :], agg[:, :], AF.Identity, scale=yse[:, :], bias=b3y[:, :]
                )

                # spatial attention: avg + max over channels, conv7, sigmoid
                rowa = postp.tile([1, N + 6], F32, tag="rowa")
                rowm = postp.tile([1, N + 6], F32, tag="rowm")
                for t in (rowa, rowm):
                    nc.vector.memset(t[:, 0:3], 0.0)
                    nc.vector.memset(t[:, N + 3 : N + 6], 0.0)
                for q in range(4):
                    sl = slice(512 * q, 512 * (q + 1))
                    ap_ = bigp.tile([1, 512], F32, tag="big")
                    nc.tensor.matmul(
                        ap_[:, :],
                        lhsT=ones64f[:, :],
                        rhs=x_se[:, sl],
                    )
                    nc.scalar.activation(
                        rowa[:, 512 * q + 3 : 512 * (q + 1) + 3], ap_[:, :], AF.Copy
                    )
                # transpose x_se blocks once; reused for channel-max and final out
                xsb = postp.tile([128, NBLK, C], F32, tag="xsb")
                for g in range(4):
                    xp = bigp.tile([128, 256], F32, tag="big")
                    for t in range(4):
                        i = 4 * g + t
                        nc.tensor.transpose(
                            out=xp[:, 64 * t : 64 * (t + 1)],
                            in_=x_se[:, 128 * i : 128 * (i + 1)],
                            identity=ident[0:C, 0:C],
                        )
                    nc.scalar.activation(
                        xsb[:, 4 * g : 4 * (g + 1), :].rearrange("p a c -> p (a c)"),
                        xp[:, :],
                        AF.Copy,
                    )
                mcol = postp.tile([128, NBLK], F32, tag="mcol")
                for i in range(NBLK):
                    nc.vector.reduce_max(mcol[:, i : i + 1], xsb[:, i, :], axis=AX.X)
                mrow_dr = vdrp.tile([N], F32)
                nc.sync.dma_start(
                    mrow_dr[:].rearrange("(k p) -> p k", p=128), mcol[:, :]
                )
                nc.sync.dma_start(rowm[:, 3 : N + 3], mrow_dr[:])
                im2c = postp.tile([14, N], F32, tag="im2c")
                for row, t in ((0, rowa), (7, rowm)):
                    for tt in range(7):
                        nc.sync.dma_start(
                            im2c[row + tt : row + tt + 1, :], t[0:1, tt : tt + N]
                        )
                attp_t = postp.tile([1, N + 6], F32, tag="rowa")
                attp = attp_t[:, 0:N]
                for q in range(4):
                    sl = slice(512 * q, 512 * (q + 1))
                    cp = bigp.tile([1, 512], F32, tag="big")
                    nc.tensor.matmul(cp[:, :], lhsT=saw[:, :], rhs=im2c[:, sl])
                    nc.scalar.activation(attp[:, sl], cp[:, :], AF.Copy)
                atc_p = bigp.tile([128, NBLK], F32, tag="big")
                for i in range(NBLK):
                    nc.tensor.transpose(
                        out=atc_p[:, i : i + 1],
                        in_=attp[:, 128 * i : 128 * (i + 1)],
                        identity=ident[0:1, 0:1],
                    )
                atc = postp.tile([128, NBLK], F32, tag="atc")
                nc.scalar.activation(atc[:, :], atc_p[:, :], AF.Sigmoid)

                # final: out = transpose(x_se)*att + x
                fo = fop.tile([128, NBLK, C], F32)
                for i in range(NBLK):
                    nc.vector.scalar_tensor_tensor(
                        fo[:, i, :],
                        in0=xsb[:, i, :],
                        scalar=atc[:, i : i + 1],
                        in1=xe[:, i, :],
                        op0=ALU.mult,
                        op1=ALU.add,
                    )
                nc.sync.dma_start(
                    out_d[b * N : (b + 1) * N, :].rearrange("(k p) c -> p k c", p=128),
                    fo[:, :, :],
                )
    nc.compile()
    return nc


_NC_CACHE = {}


def get_nc():
    if "nc" not in _NC_CACHE:
        _NC_CACHE["nc"] = build()
    return _NC_CACHE["nc"]


def prep_weights(W1, b1, W2, b2, W3, b3, se_w1, se_w2, sa_w):
    W1a, W1b = W1[:C], W1[C:]
    w1d = np.concatenate([(W1a - W1b) / 2.0, b1[None, :]], axis=0).astype(np.float32)
    w1b_ = np.concatenate([W1b / 2.0, np.zeros((1, C), np.float32)], axis=0).astype(
        np.float32
    )
    w1db = np.concatenate([w1d, w1b_], axis=1).astype(np.float32)
    saw = np.concatenate([sa_w[0, 0] / C, sa_w[0, 1]]).astype(np.float32)[:, None]
    return {
        "w1db": w1db,
        "w2": W2.astype(np.float32),
        "w3": W3.astype(np.float32),
        "b2": b2.astype(np.float32)[:, None],
        "b3": b3.astype(np.float32)[:, None],
        "b3n": (b3 * N).astype(np.float32)[:, None],
        "se1": (se_w1 / N).astype(np.float32),
        "se2": se_w2.astype(np.float32),
        "saw": saw,
    }


def kernel(x, batch, batch_size, W1, b1, W2, b2, W3, b3, se_w1, se_w2, sa_w, **kw):
    from concourse.bass_utils import run_bass_kernel_spmd

    x = np.asarray(x, np.float32)
    wts = prep_weights(
        np.asarray(W1, np.float32),
        np.asarray(b1, np.float32),
        np.asarray(W2, np.float32),
        np.asarray(b2, np.float32),
        np.asarray(W3, np.float32),
        np.asarray(b3, np.float32),
        np.asarray(se_w1, np.float32),
        np.asarray(se_w2, np.float32),
        np.asarray(sa_w, np.float32),
    )
    nc = get_nc()
    xr = x.reshape(B, N, C)
    in_maps = []
    for c in range(NCORES):
        m = {"x": np.ascontiguousarray(xr[c * NB : (c + 1) * NB].reshape(NB * N, C))}
        m.update(wts)
        in_maps.append(m)
    res = run_bass_kernel_spmd(nc, in_maps, core_ids=list(range(NCORES)))
    out = np.concatenate([r["out"] for r in res.results], axis=0)
    return out.astype(np.float32)


if __name__ == "__main__":
    nc = build()
    print("built ok")


# revision 19
# speedup vs baseline: 1.1780x; 1.0159x over previous
"""AttEdgeConv (dynamic kNN EdgeConv + SE + spatial attention) Trainium2 kernel.

Data-parallel over the 16 point clouds: 8 NeuronCores x 2 clouds each.
Per cloud (N=2048 points, C=64 channels, K=20 neighbors):
  scores s_ij = 2*x_i.x_j - |x_j|^2  (row-monotone with -dist)  via PE matmul (f32r)
  top-20 per row via DVE max8/max_index/match_replace rounds
  edge MLP: pre1 = U_i + V_j; V gathered bf16 via indirect DMA, transposed on PE
  with accumulated U^T (bf16); W2/W3 block-diagonal bf16 on PE; max over K on DVE
  straight out of PSUM; SE + conv attention on Scalar/PE.
"""

import numpy as np

import concourse.bass as bass
import concourse.mybir as mybir
from concourse import bacc
from concourse.masks import make_identity
from concourse.tile import TileContext

B, N, C, K = 16, 2048, 64, 20
NB = 2  # clouds per core
NCORES = 8
NBLK = N // 128  # 16 point-blocks per cloud
KP = K // 2  # 10 k-pair planes
F32 = mybir.dt.float32
F32R = mybir.dt.float32r
BF16 = mybir.dt.bfloat16
U32 = mybir.dt.uint32
AF = mybir.ActivationFunctionType
ALU = mybir.AluOpType
AX = mybir.AxisListType
NEG = -3.0e38


def build(nb=NB, debug=False):
    nc = bacc.Bacc("TRN2", target_bir_lowering=False)
    x_in = nc.dram_tensor("x", [nb * N, C], F32, kind="ExternalInput")
    # w1db cols 0:64 = (W1a - W1b)/2 (+b1 in row 64), cols 64:128 = W1b/2
    w1db_d = nc.dram_tensor("w1db", [C + 1, 2 * C], F32, kind="ExternalInput")
    w2_d = nc.dram_tensor("w2", [C, C], F32, kind="ExternalInput")
    w3_d = nc.dram_tensor("w3", [C, C], F32, kind="ExternalInput")
    b2_d = nc.dram_tensor("b2", [C, 1], F32, kind="ExternalInput")
    b3_d = nc.dram_tensor("b3", [C, 1], F32, kind="ExternalInput")
    b3n_d = nc.dram_tensor("b3n", [C, 1], F32, kind="ExternalInput")  # b3 * N
    se1_d = nc.dram_tensor("se1", [C, C // 16], F32, kind="ExternalInput")  # pre /N
    se2_d = nc.dram_tensor("se2", [C // 16, C], F32, kind="ExternalInput")
    saw_d = nc.dram_tensor("saw", [14, 1], F32, kind="ExternalInput")  # avg/64, max
    out_d = nc.dram_tensor("out", [nb * N, C], F32, kind="ExternalOutput")
    if debug:
        dbg = {
            "s0": nc.dram_tensor("d_s0", [128, N], F32, kind="ExternalOutput"),
            "idx0": nc.dram_tensor("d_idx0", [128, 24], U32, kind="ExternalOutput"),
            "tg0": nc.dram_tensor("d_tg0", [128, K * C], F32, kind="ExternalOutput"),
            "h10": nc.dram_tensor("d_h10", [128, KP * 128], BF16, kind="ExternalOutput"),
            "agg": nc.dram_tensor("d_agg", [C, N], F32, kind="ExternalOutput"),
        }

    with TileContext(nc) as tc:
        with (
            tc.tile_pool(name="const", bufs=1) as constp,
            tc.tile_pool(name="xe", bufs=2) as xep,
            tc.tile_pool(name="xt", bufs=2) as xtp,
            tc.tile_pool(name="xt2", bufs=1) as xt2p,
            tc.tile_pool(name="uv", bufs=2) as uvp,
            tc.tile_pool(name="ssb", bufs=2) as ssbp,
            tc.tile_pool(name="idx", bufs=3) as idxp,
            tc.tile_pool(name="gath", bufs=3) as gathp,
            tc.tile_pool(name="h", bufs=2) as hp,
            tc.tile_pool(name="post", bufs=1) as postp,
            tc.tile_pool(name="fo", bufs=1) as fop,
            tc.tile_pool(name="hmp", bufs=1) as hmp,
            tc.tile_pool(name="big_ps", bufs=2, space="PSUM") as bigp,
            tc.tile_pool(name="tp_ps", bufs=2, space="PSUM") as tpp,
            tc.tile_pool(name="mlp_ps", bufs=2, space="PSUM") as mlpp,
            tc.tile_pool(name="vdr", bufs=2, space="DRAM") as vdrp,
        ):
            # ---- constants ----
            ident = constp.tile([128, 128], F32)
            make_identity(nc, ident[:, :])
            identb = constp.tile([128, 128], BF16)
            nc.vector.tensor_copy(identb[:, :], ident[:, :])
            w1db = constp.tile([C + 1, 2 * C], F32)
            nc.sync.dma_start(w1db[:, :], w1db_d[:, :])
            # block-diagonal bf16 W2/W3: rows 0:64/cols 0:64 and rows 64:128/cols 64:128
            w2f = constp.tile([C, C], F32)
            nc.sync.dma_start(w2f[:, :], w2_d[:, :])
            w3f = constp.tile([C, C], F32)
            nc.sync.dma_start(w3f[:, :], w3_d[:, :])
            w2bd = constp.tile([128, 128], BF16)
            nc.vector.memset(w2bd[:, :], 0.0)
            nc.scalar.activation(w2bd[0:C, 0:C], w2f[:, :], AF.Copy)
            nc.scalar.activation(w2bd[C:128, C:128], w2f[:, :], AF.Copy)
            w3bd = constp.tile([128, 128], BF16)
            nc.vector.memset(w3bd[:, :], 0.0)
            nc.scalar.activation(w3bd[0:C, 0:C], w3f[:, :], AF.Copy)
            nc.scalar.activation(w3bd[C:128, C:128], w3f[:, :], AF.Copy)
            se1 = constp.tile([C, C // 16], F32)
            nc.sync.dma_start(se1[:, :], se1_d[:, :])
            se2 = constp.tile([C // 16, C], F32)
            nc.sync.dma_start(se2[:, :], se2_d[:, :])
            saw = constp.tile([14, 1], F32)
            nc.sync.dma_start(saw[:, :], saw_d[:, :])
            b2dup = constp.tile([128, 1], F32)
            nc.sync.dma_start(b2dup[0:C, :], b2_d[:, :])
            nc.sync.dma_start(b2dup[C:128, :], b2_d[:, :])
            b3c = constp.tile([C, 1], F32)
            nc.sync.dma_start(b3c[:, :], b3_d[:, :])
            b3n = constp.tile([C, 1], F32)
            nc.sync.dma_start(b3n[:, :], b3n_d[:, :])
            ones64 = constp.tile([C, 1], F32)
            nc.vector.memset(ones64[:, :], 1.0)
            ones64f = ones64
            onesrow = constp.tile([1, N], F32)
            nc.vector.memset(onesrow[:, :], 1.0)

            for b in range(nb):
                # ================= PREP =================
                xb = x_in[b * N : (b + 1) * N, :]
                xe = xep.tile([128, NBLK, C], F32)  # [pt, blk, ch]
                nc.sync.dma_start(xe[:, :, :], xb.rearrange("(k p) c -> p k c", p=128))

                # transpose pairs of point-blocks -> XT [65, 2048] variants
                xtl = xtp.tile([C + 1, N], F32, tag="xtl")  # 2*x^T ; row64 = 1
                xtr = xtp.tile([C + 1, N], F32, tag="xtr")  # x^T ; row64 = -sq
                xt2 = xt2p.tile([C, N], F32, tag="xt2")  # x^T squared
                for h in range(2):  # halves of 8 blocks
                    pt = bigp.tile([128, 512], F32, tag="big")
                    for j in range(4):
                        pr = 8 * h + 2 * j
                        nc.tensor.transpose(
                            out=pt[:, 128 * j : 128 * (j + 1)],
                            in_=xe[:, pr : pr + 2, :].rearrange("p a c -> p (a c)"),
                            identity=ident[:, :],
                        )
                    # even blocks from partitions 0:64, odd from 64:128
                    for par, off in ((0, 0), (64, 1)):
                        src = pt[par : par + 64, :].rearrange("c (j n) -> c j n", n=128)
                        dstl = xtl[0:C, 1024 * h : 1024 * (h + 1)].rearrange(
                            "c (j t n) -> c j t n", j=4, t=2
                        )[:, :, off, :]
                        dstr = xtr[0:C, 1024 * h : 1024 * (h + 1)].rearrange(
                            "c (j t n) -> c j t n", j=4, t=2
                        )[:, :, off, :]
                        nc.scalar.activation(dstl, src, AF.Copy, scale=2.0)
                        nc.scalar.activation(dstr, src, AF.Copy)
                nc.scalar.activation(xtl[C : C + 1, :], onesrow[:, :], AF.Copy)
                # squares -> -sq into xtr row 64
                nc.scalar.activation(xt2[:, :], xtr[0:C, :], AF.Square)
                for q in range(4):
                    sl = slice(512 * q, 512 * (q + 1))
                    sqp = bigp.tile([1, 512], F32, tag="big")
                    nc.tensor.matmul(
                        sqp[:, :],
                        lhsT=ones64[:, :],
                        rhs=xt2[:, sl],
                    )
                    nc.scalar.activation(xtr[C : C + 1, sl], sqp[:, :], AF.Copy, scale=-1.0)

                # U (dup bf16) per block -> udup ; V (dup bf16) staged to DRAM
                udup = uvp.tile([128, NBLK, 2 * C], F32)
                vst = uvp.tile([128, NBLK, C], F32, tag="vst")
                for i in range(NBLK):
                    up = bigp.tile([128, 2 * C], F32, tag="big")
                    lhs = xtl[:, 128 * i : 128 * (i + 1)]
                    nc.tensor.matmul(
                        up[:, :], lhsT=lhs, rhs=w1db[:, :]
                    )
                    nc.scalar.activation(
                        udup[:, i, :].rearrange("p (a c) -> p a c", a=2),
                        up[:, 0:C].rearrange("p (a c) -> p a c", a=1).to_broadcast(
                            [128, 2, C]
                        ),
                        AF.Copy,
                    )
                    nc.scalar.activation(vst[:, i, :], up[:, C : 2 * C], AF.Copy)
                v_dr = vdrp.tile([N, C], F32)
                nc.sync.dma_start(
                    v_dr[:, :].rearrange("(k p) c -> p k c", p=128), vst[:, :, :]
                )
                agg = postp.tile([C, N], F32, tag="agg")  # max-aggregated h3 (no b3)

                # ====== PER BLOCK: 3-stage skewed pipeline ======
                # iteration k issues: scores(k+2) | topk+wrap+gather(k+1) | MLP(k)

                def front_scores(i):
                    s_sb = ssbp.tile([128, N], F32)
                    for h in range(2):
                        sp = bigp.tile([128, 1024], F32, tag="big")
                        for q in range(2):
                            sl = slice(1024 * h + 512 * q, 1024 * h + 512 * (q + 1))
                            nc.tensor.matmul(
                                sp[:, 512 * q : 512 * (q + 1)],
                                lhsT=xtl[:, 128 * i : 128 * (i + 1)],
                                rhs=xtr[:, sl],
                            )
                        nc.scalar.activation(
                            s_sb[:, 1024 * h : 1024 * (h + 1)], sp[:, :], AF.Copy
                        )
                    return s_sb

                def front_topk_gather(i, s_sb):
                    mt = idxp.tile([128, 24], F32, tag="mt")
                    idx = idxp.tile([128, 24], U32, tag="idx")
                    for r in range(3):
                        sl = slice(8 * r, 8 * (r + 1))
                        nc.vector.max(mt[:, sl], s_sb[:, :])
                        nc.vector.max_index(idx[:, sl], mt[:, sl], s_sb[:, :])
                        if r < 2:
                            nc.vector.match_replace(s_sb[:, :], mt[:, sl], s_sb[:, :], NEG)
                    idxf = idxp.tile([128, K], F32, tag="idxf")
                    nc.vector.tensor_copy(idxf[:, :], idx[:, 0:K])
                    ip_ps = bigp.tile([K, 128], F32, tag="big")
                    nc.tensor.transpose(
                        out=ip_ps[:, :], in_=idxf[:, :], identity=ident[:, :]
                    )
                    idxt = idxp.tile([K, 128], F32, tag="idxt")
                    nc.scalar.activation(idxt[:, :], ip_ps[:, :], AF.Copy)
                    wta = idxp.tile([128, 16], F32, tag="wta")
                    wtb = idxp.tile([32, 16], F32, tag="wtb")
                    idxt_v = idxt[:, :].rearrange("p (a q) -> p a q", q=16)
                    nc.sync.dma_start(wta[:, :], idxt_v[0:16])
                    nc.sync.dma_start(wtb[:, :], idxt_v[16:K])
                    w_ps = bigp.tile([16, 160], F32, tag="big")
                    nc.tensor.transpose(
                        out=w_ps[:, 0:128], in_=wta[:, :], identity=ident[:, :]
                    )
                    nc.tensor.transpose(
                        out=w_ps[:, 128:160], in_=wtb[:, :], identity=ident[0:32, 0:32]
                    )
                    w16 = gathp.tile([128, 160], mybir.dt.int16, tag="w16")
                    nc.vector.tensor_copy(w16[0:16, :], w_ps[:, :])
                    nc.sync.dma_start(w16[16:32, :], w16[0:16, :])
                    nc.sync.dma_start(w16[32:64, :], w16[0:32, :])
                    nc.sync.dma_start(w16[64:128, :], w16[0:64, :])
                    tg = gathp.tile([128, K, C], F32)
                    nc.gpsimd.dma_gather(
                        out_ap=tg[:, :, :],
                        in_ap=v_dr[:, :],
                        idxs_ap=w16[:, :],
                        num_idxs=128 * K,
                        num_idxs_reg=128 * K,
                        elem_size=C,
                        single_packet=False,
                    )
                    return tg

                def mlp_block(i, tg):
                    ud = udup[:, i, :]
                    h1 = hp.tile([128, KP * 128], BF16, tag="h1")
                    for g in range(5):
                        tp = tpp.tile([128, 256], F32)
                        for t in range(2):
                            j = 2 * g + t
                            csl = slice(128 * t, 128 * (t + 1))
                            nc.tensor.matmul(
                                tp[:, csl],
                                lhsT=tg[:, 2 * j : 2 * (j + 1), :].rearrange(
                                    "p a c -> p (a c)"
                                ),
                                rhs=ident[:, :],
                                is_transpose=True,
                                start=True,
                                stop=False,
                            )
                            nc.tensor.matmul(
                                tp[:, csl],
                                lhsT=ud,
                                rhs=ident[:, :],
                                is_transpose=True,
                                start=False,
                                stop=True,
                            )
                        nc.scalar.activation(
                            h1[:, 256 * g : 256 * (g + 1)], tp[:, :], AF.Relu
                        )
                    h2 = hp.tile([128, KP * 128], BF16, tag="h2")
                    h3e = hp.tile([C, KP * 128], F32, tag="h3e")
                    h3o = hp.tile([C, KP * 128], F32, tag="h3o")
                    for c0, cw in ((0, 512), (512, 512), (1024, 256)):
                        sl = slice(c0, c0 + cw)
                        p2 = mlpp.tile([128, 512], F32, tag="mlp")
                        nc.tensor.matmul(p2[:, 0:cw], lhsT=w2bd[:, :], rhs=h1[:, sl])
                        nc.scalar.activation(
                            h2[:, sl], p2[:, 0:cw], AF.Relu, bias=b2dup[:, :]
                        )
                        p3 = mlpp.tile([128, 512], F32, tag="mlp")
                        nc.tensor.matmul(p3[:, 0:cw], lhsT=w3bd[:, :], rhs=h2[:, sl])
                        nc.scalar.activation(h3e[:, sl], p3[0:C, 0:cw], AF.Copy)
                        nc.scalar.activation(h3o[:, sl], p3[C:128, 0:cw], AF.Copy)
                    return h3e, h3o

                def agg_block(i, h3e, h3o):
                    # deferred one iteration so these DVE ops never stall the queue
                    hm = hmp.tile([C, KP * 128], F32, tag="hm")
                    nc.vector.tensor_tensor(hm[:, :], h3e[:, :], h3o[:, :], op=ALU.max)
                    nc.vector.reduce_max(
                        agg[:, 128 * i : 128 * (i + 1)],
                        hm[:, :].rearrange("c (j n) -> c n j", j=KP),
                        axis=AX.X,
                    )

                ssb_d = {0: front_scores(0), 1: front_scores(1)}
                tg_d = {0: front_topk_gather(0, ssb_d.pop(0))}
                h3_d = {}
                for i in range(NBLK):
                    if i + 2 < NBLK:
                        ssb_d[i + 2] = front_scores(i + 2)
                    if i + 1 < NBLK:
                        tg_d[i + 1] = front_topk_gather(i + 1, ssb_d.pop(i + 1))
                    h3_d[i] = mlp_block(i, tg_d.pop(i))
                    if i - 1 in h3_d:
                        agg_block(i - 1, *h3_d.pop(i - 1))
                agg_block(NBLK - 1, *h3_d.pop(NBLK - 1))

                # ================= POST =================
                if debug and b == 0:
                    nc.sync.dma_start(dbg["agg"][:, :], agg[:, :])
                # SE channel attention: s = sum(agg)/N + b3 ; se1 pre-divided by N
                s_sum = postp.tile([C, 1], F32, tag="ssum")
                nc.vector.reduce_sum(s_sum[:, :], agg[:, :], axis=AX.X)
                s_in = postp.tile([C, 1], F32, tag="sin")
                nc.vector.tensor_add(s_in[:, :], s_sum[:, :], b3n[:, :])
                se_p = bigp.tile([C // 16, 1], F32, tag="big")
                nc.tensor.matmul(se_p[:, :], lhsT=se1[:, :], rhs=s_in[:, :])
                se_t = postp.tile([C // 16, 1], F32, tag="set")
                nc.scalar.activation(se_t[:, :], se_p[:, :], AF.Relu)
                se_p2 = bigp.tile([C, 1], F32, tag="big")
                nc.tensor.matmul(se_p2[:, :], lhsT=se2[:, :], rhs=se_t[:, :])
                yse = postp.tile([C, 1], F32, tag="yse")
                nc.scalar.activation(yse[:, :], se_p2[:, :], AF.Sigmoid)
                # x_se = (agg + b3) * yse  == agg*yse + (b3*yse)  on Scalar
                b3y = postp.tile([C, 1], F32, tag="b3y")
                nc.vector.tensor_mul(b3y[:, :], b3c[:, :], yse[:, :])
                x_se = postp.tile([C, N], F32, tag="xse")
                nc.scalar.activation(
                    x_se[:, :], agg[:, :], AF.Identity, scale=yse[:, :], bias=b3y[:, :]
                )

                # spatial attention: avg + max over channels, conv7, sigmoid
                rowa = postp.tile([1, N + 6], F32, tag="rowa")
                rowm = postp.tile([1, N + 6], F32, tag="rowm")
                for t in (rowa, rowm):
                    nc.vector.memset(t[:, 0:3], 0.0)
                    nc.vector.memset(t[:, N + 3 : N + 6], 0.0)
                for q in range(4):
                    sl = slice(512 * q, 512 * (q + 1))
                    ap_ = bigp.tile([1, 512], F32, tag="big")
                    nc.tensor.matmul(
                        ap_[:, :],
                        lhsT=ones64f[:, :],
                        rhs=x_se[:, sl],
                    )
                    nc.scalar.activation(
                        rowa[:, 512 * q + 3 : 512 * (q + 1) + 3], ap_[:, :], AF.Copy
                    )
                # transpose x_se blocks once; reused for channel-max and final out
                xsb = postp.tile([128, NBLK, C], F32, tag="xsb")
                for g in range(4):
                    xp = bigp.tile([128, 256], F32, tag="big")
                    for t in range(4):
                        i = 4 * g + t
                        nc.tensor.transpose(
                            out=xp[:, 64 * t : 64 * (t + 1)],
                            in_=x_se[:, 128 * i : 128 * (i + 1)],
                            identity=ident[0:C, 0:C],
                        )
                    nc.scalar.activation(
                        xsb[:, 4 * g : 4 * (g + 1), :].rearrange("p a c -> p (a c)"),
                        xp[:, :],
                        AF.Copy,
                    )
                mcol = postp.tile([128, NBLK], F32, tag="mcol")
                for i in range(NBLK):
                    nc.vector.reduce_max(mcol[:, i : i + 1], xsb[:, i, :], axis=AX.X)
                mrow_dr = vdrp.tile([N], F32)
                nc.sync.dma_start(
                    mrow_dr[:].rearrange("(k p) -> p k", p=128), mcol[:, :]
                )
                nc.sync.dma_start(rowm[:, 3 : N + 3], mrow_dr[:])
                im2c = postp.tile([14, N], F32, tag="im2c")
                for row, t in ((0, rowa), (7, rowm)):
                    for tt in range(7):
                        nc.sync.dma_start(
                            im2c[row + tt : row + tt + 1, :], t[0:1, tt : tt + N]
                        )
                attp_t = postp.tile([1, N + 6], F32, tag="rowa")
                attp = attp_t[:, 0:N]
                for q in range(4):
                    sl = slice(512 * q, 512 * (q + 1))
                    cp = bigp.tile([1, 512], F32, tag="big")
                    nc.tensor.matmul(cp[:, :], lhsT=saw[:, :], rhs=im2c[:, sl])
                    nc.scalar.activation(attp[:, sl], cp[:, :], AF.Copy)
                atc_p = bigp.tile([128, NBLK], F32, tag="big")
                for i in range(NBLK):
                    nc.tensor.transpose(
                        out=atc_p[:, i : i + 1],
                        in_=attp[:, 128 * i : 128 * (i + 1)],
                        identity=ident[0:1, 0:1],
                    )
                atc = postp.tile([128, NBLK], F32, tag="atc")
                nc.scalar.activation(atc[:, :], atc_p[:, :], AF.Sigmoid)

                # final: out = transpose(x_se)*att + x
                fo = fop.tile([128, NBLK, C], F32)
                for i in range(NBLK):
                    nc.vector.scalar_tensor_tensor(
                        fo[:, i, :],
                        in0=xsb[:, i, :],
                        scalar=atc[:, i : i + 1],
                        in1=xe[:, i, :],
                        op0=ALU.mult,
                        op1=ALU.add,
                    )
                nc.sync.dma_start(
                    out_d[b * N : (b + 1) * N, :].rearrange("(k p) c -> p k c", p=128),
                    fo[:, :, :],
                )
    nc.compile()
    return nc


_NC_CACHE = {}


def get_nc():
    if "nc" not in _NC_CACHE:
        _NC_CACHE["nc"] = build()
    return _NC_CACHE["nc"]


def prep_weights(W1, b1, W2, b2, W3, b3, se_w1, se_w2, sa_w):
    W1a, W1b = W1[:C], W1[C:]
    w1d = np.concatenate([(W1a - W1b) / 2.0, b1[None, :]], axis=0).astype(np.float32)
    w1b_ = np.concatenate([W1b / 2.0, np.zeros((1, C), np.float32)], axis=0).astype(
        np.float32
    )
    w1db = np.concatenate([w1d, w1b_], axis=1).astype(np.float32)
    saw = np.concatenate([sa_w[0, 0] / C, sa_w[0, 1]]).astype(np.float32)[:, None]
    return {
        "w1db": w1db,
        "w2": W2.astype(np.float32),
        "w3": W3.astype(np.float32),
        "b2": b2.astype(np.float32)[:, None],
        "b3": b3.astype(np.float32)[:, None],
        "b3n": (b3 * N).astype(np.float32)[:, None],
        "se1": (se_w1 / N).astype(np.float32),
        "se2": se_w2.astype(np.float32),
        "saw": saw,
    }


def kernel(x, batch, batch_size, W1, b1, W2, b2, W3, b3, se_w1, se_w2, sa_w, **kw):
    from concourse.bass_utils import run_bass_kernel_spmd

    x = np.asarray(x, np.float32)
    wts = prep_weights(
        np.asarray(W1, np.float32),
        np.asarray(b1, np.float32),
        np.asarray(W2, np.float32),
        np.asarray(b2, np.float32),
        np.asarray(W3, np.float32),
        np.asarray(b3, np.float32),
        np.asarray(se_w1, np.float32),
        np.asarray(se_w2, np.float32),
        np.asarray(sa_w, np.float32),
    )
    nc = get_nc()
    xr = x.reshape(B, N, C)
    in_maps = []
    for c in range(NCORES):
        m = {"x": np.ascontiguousarray(xr[c * NB : (c + 1) * NB].reshape(NB * N, C))}
        m.update(wts)
        in_maps.append(m)
    res = run_bass_kernel_spmd(nc, in_maps, core_ids=list(range(NCORES)))
    out = np.concatenate([r["out"] for r in res.results], axis=0)
    return out.astype(np.float32)


if __name__ == "__main__":
    nc = build()
    print("built ok")
